# revision 13
# baseline (speedup 1.0000x reference)
"""AnchorDetector Trainium2 kernel (8-core SPMD, self-contained).

Strategy
--------
Shapes are fixed: hidden [4, 8192, 1024] f32, W [1024] f32, b scalar.

Sharding: 8 shards = 4 batches x 2 T-halves (4096 timesteps each). The host
pre-transposes each shard to [1024, 4097] (D on partitions, T on the free
axis, one halo column for the t-1 difference) so that both per-timestep
D-reductions become TensorE matmuls with tiny stationary operands:

  prior[t] = sum_d h[d,t] * W[d]        lhsT = [W_c | 0]   rhs = h chunk
  ss[t]    = sum_d (h[d,t]-h[d,t-1])^2  lhsT = [0 | ones]  rhs = square(delta)

Both accumulate over the 8 D-chunks into one shared [2, 512] PSUM bank per
T-slice (the zero column of each lhsT makes the writes disjoint by row).
VectorE only does the delta subtract; ScalarE squares (emitting the square
in SS dtype) and evacuates PSUM. The tiny [B,T] postprocessing (standardize,
sigmoid, peak detection, top-k) runs on the host in float64.

Engine budget per core (target ~47us DMA roofline): DMA 16.8MB in, V 1 pass
(~35us), S ~1 pass (~29us), PE 2 reduce streams (~27us at 1 cyc/row).
"""

import math
import numpy as np
from contextlib import ExitStack

import concourse.bass as bass
import concourse.tile as tile
from concourse import bacc, mybir
from concourse.bass_utils import run_bass_kernel_spmd

F32 = mybir.dt.float32

B, T, D = 4, 8192, 1024
NCORES = 8
HALF = T // 2  # timesteps per core
NCH = D // 128  # 8 D-chunks
SLICE = 512  # matmul free-dim per PSUM bank
NSL = HALF // SLICE  # 8 T-slices per core
PIECE = 2048  # delta/square granularity (finer -> shorter pipeline tail)
NPC = HALF // PIECE

PRIOR_W = 0.5
RUNTIME_W = 0.5
THRESHOLD = 0.6
MAX_CAND = 64

# matmul precision modes, measured on hardware vs float64 (probe_dtypes.py):
#   prior abs-err std: f32 1.4e-7 | f32r 1.3e-4 | f16 2.9e-4 | bf16 2.4e-3
#   ss    rel-err std: f32 4.7e-8 | f32r 5.8e-6 | f16 1.1e-5 | bf16 9.2e-5
# f32r streams at 1 cyc/row (vs 4 for f32) and keeps total score noise
# ~1.3e-5, a 5x margin under the smallest top-64 boundary gap (9.7e-5).
PRIOR_MODE = "f32r"  # "f32" (exact, 4 cyc/row) | "f32r" (1 cyc/row)
SS_MODE = "f32r"     # dtype of squared deltas fed to the ss matmul

_CACHE = {}

# set by test.py to capture a neuron-profile trace of the SPMD run
TRACE = False
LAST_EXEC_NS = None
LAST_RESULTS = None


def _build():
    ss_dt = {"f16": mybir.dt.float16, "bf16": mybir.dt.bfloat16, "f32": F32,
             "f32r": mybir.dt.float32r}[SS_MODE]
    pr_dt = {"f32": F32, "f32r": mybir.dt.float32r}[PRIOR_MODE]

    nc = bacc.Bacc("TRN2", target_bir_lowering=False, debug=False)
    # hidden shard, transposed, typed in the prior matmul dtype (f32r is the
    # same 32-bit payload; the V/S passes bitcast back to plain f32)
    ht = nc.dram_tensor("ht", [D, HALF + 1], pr_dt, kind="ExternalInput").ap()
    # [W_c | 0] interleaved columns for the prior lhsT
    wz = nc.dram_tensor("wz", [128, 2 * NCH], pr_dt, kind="ExternalInput").ap()
    # [0 | 1] columns for the ss lhsT
    oz = nc.dram_tensor("oz", [128, 2], ss_dt, kind="ExternalInput").ap()
    out = nc.dram_tensor("out", [2, HALF], F32, kind="ExternalOutput").ap()

    with tile.TileContext(nc) as tc, ExitStack() as ctx:
        data = ctx.enter_context(tc.tile_pool(name="data", bufs=1))
        work = ctx.enter_context(tc.tile_pool(name="work", bufs=3))
        sqp = ctx.enter_context(tc.tile_pool(name="sqp", bufs=3))
        consts = ctx.enter_context(tc.tile_pool(name="consts", bufs=1))
        psum = ctx.enter_context(tc.tile_pool(name="psum", bufs=1, space="PSUM"))
        resp = ctx.enter_context(tc.tile_pool(name="resp", bufs=1))

        w_sb = consts.tile([128, 2 * NCH], pr_dt, tag="w", name="w_sb")
        nc.sync.dma_start(w_sb[:], wz[:])
        onesz = consts.tile([128, 2], ss_dt, tag="onesz", name="onesz")
        nc.sync.dma_start(onesz[:], oz[:])



        ps = [
            psum.tile([2, SLICE], F32, tag=f"ps{s}", name=f"ps{s}")
            for s in range(NSL)
        ]

        chunks = []
        for c in range(NCH):
            t = data.tile([128, HALF + 1], pr_dt, tag=f"ch{c}", name=f"ch{c}")
            nc.sync.dma_start(t[:], ht[c * 128 : (c + 1) * 128, :])
            chunks.append(t)

        for c in range(NCH):
            t = chunks[c]
            tf = t[:].bitcast(F32)  # exact f32 view for the V/S passes
            for p in range(NPC):
                po = p * PIECE
                delta = work.tile([128, PIECE], F32, tag="delta", name=f"d{c}_{p}")
                nc.vector.tensor_tensor(
                    delta[:],
                    tf[:, 1 + po : 1 + po + PIECE],
                    tf[:, po : po + PIECE],
                    mybir.AluOpType.subtract,
                )
                sq = sqp.tile([128, PIECE], ss_dt, tag="sq", name=f"sq{c}_{p}")
                nc.scalar.activation(
                    sq[:], delta[:], mybir.ActivationFunctionType.Square
                )
                for si in range(PIECE // SLICE):
                    s = p * (PIECE // SLICE) + si
                    lo = s * SLICE
                    nc.tensor.matmul(
                        ps[s][:],
                        w_sb[:, 2 * c : 2 * c + 2],
                        t[:, 1 + lo : 1 + lo + SLICE],
                        start=(c == 0),
                        stop=False,
                    )
                    nc.tensor.matmul(
                        ps[s][:],
                        onesz[:],
                        sq[:, si * SLICE : (si + 1) * SLICE],
                        start=False,
                        stop=(c == NCH - 1),
                    )

        for s in range(NSL):
            r = resp.tile([2, SLICE], F32, tag="ret", name=f"ret{s}", bufs=2)
            nc.scalar.copy(r[:], ps[s][:])
            nc.sync.dma_start(out[:, s * SLICE : (s + 1) * SLICE], r[:])

    nc.compile()
    return nc


def _get_nc():
    key = (PRIOR_MODE, SS_MODE)
    if key not in _CACHE:
        _CACHE[key] = _build()
    return _CACHE[key]


def _make_shards(hidden, W):
    wz = np.zeros((128, 2 * NCH), dtype=np.float32)
    for c in range(NCH):
        wz[:, 2 * c] = W[c * 128 : (c + 1) * 128]
    ss_np = {"f16": np.float16, "bf16": None, "f32": np.float32,
             "f32r": np.float32}[SS_MODE]
    oz = np.zeros((128, 2), dtype=ss_np)
    oz[:, 1] = 1.0
    in_maps = []
    for b in range(B):
        for h in range(2):
            start = h * HALF
            prev = hidden[b, max(start - 1, 0)]
            block = hidden[b, start : start + HALF]
            sh = np.empty((D, HALF + 1), dtype=np.float32)
            sh[:, 0] = prev
            sh[:, 1:] = block.T
            in_maps.append({"ht": sh, "wz": wz, "oz": oz})
    return in_maps


def _postprocess(prior_raw, ss, b):
    """float64 host postprocess of the per-timestep reductions."""
    prior_raw = prior_raw.astype(np.float64) + float(b)
    runtime_raw = np.sqrt(ss.astype(np.float64) / D)

    def standardize(v):
        mean = v.mean(axis=1, keepdims=True)
        std = np.maximum(v.std(axis=1, keepdims=True), 1e-6)
        return (v - mean) / std

    def sigmoid(x):
        return 1.0 / (1.0 + np.exp(-x))

    runtime_logits = standardize(runtime_raw)
    runtime_score = sigmoid(runtime_logits)
    prior_logits = standardize(prior_raw)
    prior_score = sigmoid(prior_logits)

    combined = PRIOR_W * prior_logits + RUNTIME_W * runtime_logits
    scores = sigmoid(combined)

    positions = np.arange(T, dtype=np.int32)
    span = np.stack([np.maximum(positions - 1, 0), positions], axis=-1)
    span_bounds = np.broadcast_to(span, (B, T, 2)).astype(np.int32)

    left = np.concatenate([scores[:, :1], scores[:, :-1]], axis=1)
    right = np.concatenate([scores[:, 1:], scores[:, -1:]], axis=1)
    valid = (scores >= left) & (scores >= right) & (scores >= THRESHOLD)

    masked = np.where(valid, scores, -np.inf)
    # jax.lax.top_k: descending values, ties -> lowest index first
    order = np.argsort(-masked, axis=1, kind="stable")[:, :MAX_CAND]
    top_vals = np.take_along_axis(masked, order, axis=1)
    cand_mask = top_vals > -np.inf
    cand_idx = np.sort(np.where(cand_mask, order, T), axis=-1).astype(np.int32)

    return (
        scores.astype(np.float32),
        combined.astype(np.float32),
        prior_score.astype(np.float32),
        runtime_score.astype(np.float32),
        span_bounds,
        cand_idx,
        cand_mask,
    )


def kernel(hidden, W, b):
    global LAST_EXEC_NS, LAST_RESULTS
    hidden = np.asarray(hidden, dtype=np.float32)
    W = np.asarray(W, dtype=np.float32)
    nc = _get_nc()
    in_maps = _make_shards(hidden, W)
    res = run_bass_kernel_spmd(
        nc, in_maps, core_ids=list(range(NCORES)), trace=TRACE
    )
    LAST_EXEC_NS = res.exec_time_ns
    LAST_RESULTS = res

    prior_raw = np.empty((B, T), dtype=np.float32)
    ss = np.empty((B, T), dtype=np.float32)
    for i in range(NCORES):
        bi, h = divmod(i, 2)
        o = res.results[i]["out"]
        prior_raw[bi, h * HALF : (h + 1) * HALF] = o[0]
        ss[bi, h * HALF : (h + 1) * HALF] = o[1]

    return _postprocess(prior_raw, ss, b)


# revision 31
# speedup vs baseline: 1.2801x; 1.2801x over previous
"""AnchorDetector Trainium2 kernel (8-core SPMD, self-contained).

Strategy
--------
Shapes are fixed: hidden [4, 8192, 1024] f32, W [1024] f32, b scalar.

Sharding: 8 shards = 4 batches x 2 T-halves (4096 timesteps each). The host
pre-transposes each shard to [1024, 4097] (D on partitions, T on the free
axis, one halo column for the t-1 difference) so that both per-timestep
D-reductions become TensorE matmuls with tiny stationary operands:

  prior[t] = sum_d h[d,t] * W[d]        lhsT = [W_c | 0]   rhs = h chunk
  ss[t]    = sum_d (h[d,t]-h[d,t-1])^2  lhsT = [0 | ones]  rhs = square(delta)

Both accumulate over the 8 D-chunks into one shared [2, 512] PSUM bank per
T-slice (the zero column of each lhsT makes the writes disjoint by row).
VectorE only does the delta subtract; ScalarE squares (emitting the square
in SS dtype) and evacuates PSUM. The tiny [B,T] postprocessing (standardize,
sigmoid, peak detection, top-k) runs on the host in float64.

Engine budget per core (target ~47us DMA roofline): DMA 16.8MB in, V 1 pass
(~35us), S ~1 pass (~29us), PE 2 reduce streams (~27us at 1 cyc/row).
"""

import math
import numpy as np
from contextlib import ExitStack

import concourse.bass as bass
import concourse.tile as tile
from concourse import bacc, mybir
from concourse.bass_utils import run_bass_kernel_spmd

F32 = mybir.dt.float32

B, T, D = 4, 8192, 1024
NCORES = 8
HALF = T // 2  # timesteps per core
NCH = D // 128  # 8 D-chunks
SLICE = 512  # matmul free-dim per PSUM bank
NSL = HALF // SLICE  # 8 T-slices per core
HC = 2048  # DMA half-chunk width (each half-tile carries a +1 halo col)

PRIOR_W = 0.5
RUNTIME_W = 0.5
THRESHOLD = 0.6
MAX_CAND = 64

# matmul precision modes, measured on hardware vs float64 (probe_dtypes.py):
#   prior abs-err std: f32 1.4e-7 | f32r 1.3e-4 | f16 2.9e-4 | bf16 2.4e-3
#   ss    rel-err std: f32 4.7e-8 | f32r 5.8e-6 | f16 1.1e-5 | bf16 9.2e-5
# f32r streams at 1 cyc/row (vs 4 for f32) and keeps total score noise
# ~1.3e-5, a 5x margin under the smallest top-64 boundary gap (9.7e-5).
PRIOR_MODE = "f32r"  # "f32" (exact, 4 cyc/row) | "f32r" (1 cyc/row)
SS_MODE = "f32r"     # dtype of squared deltas fed to the ss matmul

_CACHE = {}

# set by test.py to capture a neuron-profile trace of the SPMD run
TRACE = False
LAST_EXEC_NS = None
LAST_RESULTS = None


def _build():
    ss_dt = {"f16": mybir.dt.float16, "bf16": mybir.dt.bfloat16, "f32": F32,
             "f32r": mybir.dt.float32r}[SS_MODE]
    pr_dt = {"f32": F32, "f32r": mybir.dt.float32r}[PRIOR_MODE]

    nc = bacc.Bacc("TRN2", target_bir_lowering=False, debug=False)
    # hidden shard, transposed, typed in the prior matmul dtype (f32r is the
    # same 32-bit payload; the V/S passes bitcast back to plain f32)
    ht = nc.dram_tensor("ht", [D, HALF + 1], pr_dt, kind="ExternalInput").ap()
    # [W_c | 0] interleaved columns for the prior lhsT
    wz = nc.dram_tensor("wz", [128, 2 * NCH], pr_dt, kind="ExternalInput").ap()
    # [0 | 1] columns for the ss lhsT (plus an fp16 copy for GPSIMD-squared
    # pieces, which can't produce f32r)
    oz = nc.dram_tensor("oz", [128, 2], ss_dt, kind="ExternalInput").ap()
    oz16 = nc.dram_tensor("oz16", [128, 2], mybir.dt.float16,
                          kind="ExternalInput").ap()
    out = nc.dram_tensor("out", [2, HALF], F32, kind="ExternalOutput").ap()

    with tile.TileContext(nc) as tc, ExitStack() as ctx:
        data = ctx.enter_context(tc.tile_pool(name="data", bufs=1))
        work = ctx.enter_context(tc.tile_pool(name="work", bufs=6))
        sqp = ctx.enter_context(tc.tile_pool(name="sqp", bufs=4))
        consts = ctx.enter_context(tc.tile_pool(name="consts", bufs=1))
        psum = ctx.enter_context(tc.tile_pool(name="psum", bufs=1, space="PSUM"))
        resp = ctx.enter_context(tc.tile_pool(name="resp", bufs=1))

        # SWDGE for the tiny const loads so the HWDGE chunk stream starts at 0
        w_sb = consts.tile([128, 2 * NCH], pr_dt, tag="w", name="w_sb")
        nc.gpsimd.dma_start(w_sb[:], wz[:])
        onesz = consts.tile([128, 2], ss_dt, tag="onesz", name="onesz")
        nc.gpsimd.dma_start(onesz[:], oz[:])
        onesz16 = consts.tile([128, 2], mybir.dt.float16, tag="onesz16",
                              name="onesz16")
        nc.gpsimd.dma_start(onesz16[:], oz16[:])



        ps = [
            psum.tile([2, SLICE], F32, tag=f"ps{s}", name=f"ps{s}")
            for s in range(NSL)
        ]
        ret = resp.tile([2, HALF], F32, tag="ret", name="ret")

        # Slice-major streaming: the shard is DMA'd as T-blocks x 8 D-chunks,
        # so every PSUM slice finishes accumulating (and retires + stores)
        # right after its block's DMAs land. Only the final 512-col sliver's
        # sub->square->matmul->retire chain hangs off the last byte.
        # blocks: three 1024-col blocks, then two 512-col slivers.
        blocks = [(0, 1024), (1024, 1024), (2048, 1024), (3072, 512), (3584, 512)]

        tiles = {}
        for bi, (bo, bw) in enumerate(blocks):
            for c in range(NCH):
                t = data.tile(
                    [128, bw + 1], pr_dt, tag=f"b{bi}c{c}", name=f"b{bi}c{c}"
                )
                nc.sync.dma_start(
                    t[:], ht[c * 128 : (c + 1) * 128, bo : bo + bw + 1]
                )
                tiles[(bi, c)] = t

        ip = 0
        for bi, (bo, bw) in enumerate(blocks):
            last = bi == len(blocks) - 1
            for c in range(NCH):
                t = tiles[(bi, c)]
                tf = t[:].bitcast(F32)
                delta = work.tile([128, bw], F32, tag="delta", name=f"d{bi}_{c}")
                # GPSIMD (otherwise idle) fully handles every 4th piece:
                # sub + square (fp16 out), freeing V and ACT. The phase keeps
                # the final sliver on the fast V/ACT path.
                on_g = ip % 4 == 1
                ip += 1
                sub_eng = nc.gpsimd if on_g else nc.vector
                sub_eng.tensor_tensor(
                    delta[:],
                    tf[:, 1 : 1 + bw],
                    tf[:, 0:bw],
                    mybir.AluOpType.subtract,
                )
                if on_g:
                    sq = sqp.tile([128, bw], mybir.dt.float16, tag="sq",
                                  name=f"sq{bi}_{c}")
                    nc.gpsimd.tensor_tensor(
                        sq[:], delta[:], delta[:], mybir.AluOpType.mult
                    )
                else:
                    sq = sqp.tile([128, bw], ss_dt, tag="sq", name=f"sq{bi}_{c}")
                    nc.scalar.activation(
                        sq[:], delta[:], mybir.ActivationFunctionType.Square
                    )
                for si in range(bw // SLICE):
                    s = (bo + si * SLICE) // SLICE
                    slo = si * SLICE
                    nc.tensor.matmul(
                        ps[s][:],
                        w_sb[:, 2 * c : 2 * c + 2],
                        t[:, 1 + slo : 1 + slo + SLICE],
                        start=(c == 0),
                        stop=False,
                    )
                    nc.tensor.matmul(
                        ps[s][:],
                        onesz16[:] if on_g else onesz[:],
                        sq[:, slo : slo + SLICE],
                        start=False,
                        stop=(c == NCH - 1),
                    )
            # block complete: evacuate its PSUM slices and store them
            for si in range(bw // SLICE):
                s = (bo + si * SLICE) // SLICE
                dst = ret[:, s * SLICE : (s + 1) * SLICE]
                if si % 2 == 0:
                    nc.scalar.copy(dst, ps[s][:])
                else:
                    nc.vector.tensor_copy(dst, ps[s][:])
            nc.sync.dma_start(out[:, bo : bo + bw], ret[:, bo : bo + bw])

    nc.compile()
    return nc


def _get_nc():
    key = (PRIOR_MODE, SS_MODE)
    if key not in _CACHE:
        _CACHE[key] = _build()
    return _CACHE[key]


def _make_shards(hidden, W):
    wz = np.zeros((128, 2 * NCH), dtype=np.float32)
    for c in range(NCH):
        wz[:, 2 * c] = W[c * 128 : (c + 1) * 128]
    ss_np = {"f16": np.float16, "bf16": None, "f32": np.float32,
             "f32r": np.float32}[SS_MODE]
    oz = np.zeros((128, 2), dtype=ss_np)
    oz[:, 1] = 1.0
    oz16 = oz.astype(np.float16)
    in_maps = []
    for b in range(B):
        for h in range(2):
            start = h * HALF
            prev = hidden[b, max(start - 1, 0)]
            block = hidden[b, start : start + HALF]
            sh = np.empty((D, HALF + 1), dtype=np.float32)
            sh[:, 0] = prev
            sh[:, 1:] = block.T
            in_maps.append({"ht": sh, "wz": wz, "oz": oz, "oz16": oz16})
    return in_maps


def _postprocess(prior_raw, ss, b):
    """float64 host postprocess of the per-timestep reductions."""
    prior_raw = prior_raw.astype(np.float64) + float(b)
    runtime_raw = np.sqrt(ss.astype(np.float64) / D)

    def standardize(v):
        mean = v.mean(axis=1, keepdims=True)
        std = np.maximum(v.std(axis=1, keepdims=True), 1e-6)
        return (v - mean) / std

    def sigmoid(x):
        return 1.0 / (1.0 + np.exp(-x))

    runtime_logits = standardize(runtime_raw)
    runtime_score = sigmoid(runtime_logits)
    prior_logits = standardize(prior_raw)
    prior_score = sigmoid(prior_logits)

    combined = PRIOR_W * prior_logits + RUNTIME_W * runtime_logits
    scores = sigmoid(combined)

    positions = np.arange(T, dtype=np.int32)
    span = np.stack([np.maximum(positions - 1, 0), positions], axis=-1)
    span_bounds = np.broadcast_to(span, (B, T, 2)).astype(np.int32)

    left = np.concatenate([scores[:, :1], scores[:, :-1]], axis=1)
    right = np.concatenate([scores[:, 1:], scores[:, -1:]], axis=1)
    valid = (scores >= left) & (scores >= right) & (scores >= THRESHOLD)

    masked = np.where(valid, scores, -np.inf)
    # jax.lax.top_k: descending values, ties -> lowest index first
    order = np.argsort(-masked, axis=1, kind="stable")[:, :MAX_CAND]
    top_vals = np.take_along_axis(masked, order, axis=1)
    cand_mask = top_vals > -np.inf
    cand_idx = np.sort(np.where(cand_mask, order, T), axis=-1).astype(np.int32)

    return (
        scores.astype(np.float32),
        combined.astype(np.float32),
        prior_score.astype(np.float32),
        runtime_score.astype(np.float32),
        span_bounds,
        cand_idx,
        cand_mask,
    )


def kernel(hidden, W, b):
    global LAST_EXEC_NS, LAST_RESULTS
    hidden = np.asarray(hidden, dtype=np.float32)
    W = np.asarray(W, dtype=np.float32)
    nc = _get_nc()
    in_maps = _make_shards(hidden, W)
    res = run_bass_kernel_spmd(
        nc, in_maps, core_ids=list(range(NCORES)), trace=TRACE
    )
    LAST_EXEC_NS = res.exec_time_ns
    LAST_RESULTS = res

    prior_raw = np.empty((B, T), dtype=np.float32)
    ss = np.empty((B, T), dtype=np.float32)
    for i in range(NCORES):
        bi, h = divmod(i, 2)
        o = res.results[i]["out"]
        prior_raw[bi, h * HALF : (h + 1) * HALF] = o[0]
        ss[bi, h * HALF : (h + 1) * HALF] = o[1]

    return _postprocess(prior_raw, ss, b)


# revision 44
# speedup vs baseline: 1.2818x; 1.0013x over previous
"""AnchorDetector Trainium2 kernel (8-core SPMD, self-contained).

Strategy
--------
Shapes are fixed: hidden [4, 8192, 1024] f32, W [1024] f32, b scalar.

Sharding: 8 shards = 4 batches x 2 T-halves (4096 timesteps each). The host
pre-transposes each shard to [1024, 4097] (D on partitions, T on the free
axis, one halo column for the t-1 difference) so that both per-timestep
D-reductions become TensorE matmuls with tiny stationary operands:

  prior[t] = sum_d h[d,t] * W[d]        lhsT = [W_c | 0]   rhs = h chunk
  ss[t]    = sum_d (h[d,t]-h[d,t-1])^2  lhsT = [0 | ones]  rhs = square(delta)

Both accumulate over the 8 D-chunks into one shared [2, 512] PSUM bank per
T-slice (the zero column of each lhsT makes the writes disjoint by row).
VectorE only does the delta subtract; ScalarE squares (emitting the square
in SS dtype) and evacuates PSUM. The tiny [B,T] postprocessing (standardize,
sigmoid, peak detection, top-k) runs on the host in float64.

The shard streams in slice-major order (T-blocks x 8 D-chunks) so PSUM
slices retire mid-stream; GPSIMD absorbs every 4th sub+square piece. Engine
busy per core (cost model): DMA 46.9us (roofline), ACT ~30us, V ~29us,
PE ~28us, GPSIMD ~38us; simulated exec ~55us.
"""

import numpy as np
from contextlib import ExitStack

import concourse.tile as tile
from concourse import bacc, mybir
from concourse.bass_utils import run_bass_kernel_spmd

F32 = mybir.dt.float32

B, T, D = 4, 8192, 1024
NCORES = 8
HALF = T // 2  # timesteps per core
NCH = D // 128  # 8 D-chunks
SLICE = 512  # matmul free-dim per PSUM bank
NSL = HALF // SLICE  # 8 T-slices per core

PRIOR_W = 0.5
RUNTIME_W = 0.5
THRESHOLD = 0.6
MAX_CAND = 64

# matmul precision modes, measured on hardware vs float64 (probe_dtypes.py):
#   prior abs-err std: f32 1.4e-7 | f32r 1.3e-4 | f16 2.9e-4 | bf16 2.4e-3
#   ss    rel-err std: f32 4.7e-8 | f32r 5.8e-6 | f16 1.1e-5 | bf16 9.2e-5
# f32r streams at 1 cyc/row (vs 4 for f32) and keeps total score noise
# ~1.3e-5, a 5x margin under the smallest top-64 boundary gap (9.7e-5).
PRIOR_MODE = "f32r"  # "f32" (exact, 4 cyc/row) | "f32r" (1 cyc/row)
SS_MODE = "f32r"     # dtype of squared deltas fed to the ss matmul

_CACHE = {}

# set by test.py to capture a neuron-profile trace of the SPMD run
TRACE = False
LAST_EXEC_NS = None
LAST_RESULTS = None


def _build():
    ss_dt = {"f16": mybir.dt.float16, "bf16": mybir.dt.bfloat16, "f32": F32,
             "f32r": mybir.dt.float32r}[SS_MODE]
    pr_dt = {"f32": F32, "f32r": mybir.dt.float32r}[PRIOR_MODE]

    nc = bacc.Bacc("TRN2", target_bir_lowering=False, debug=False)
    # hidden shard, transposed, typed in the prior matmul dtype (f32r is the
    # same 32-bit payload; the V/S passes bitcast back to plain f32)
    ht = nc.dram_tensor("ht", [D, HALF + 1], pr_dt, kind="ExternalInput").ap()
    # [W_c | 0] interleaved columns for the prior lhsT
    wz = nc.dram_tensor("wz", [128, 2 * NCH], pr_dt, kind="ExternalInput").ap()
    # [0 | 1] columns for the ss lhsT (plus an fp16 copy for GPSIMD-squared
    # pieces, which can't produce f32r)
    oz = nc.dram_tensor("oz", [128, 2], ss_dt, kind="ExternalInput").ap()
    oz16 = nc.dram_tensor("oz16", [128, 2], mybir.dt.float16,
                          kind="ExternalInput").ap()
    out = nc.dram_tensor("out", [2, HALF], F32, kind="ExternalOutput").ap()

    with tile.TileContext(nc) as tc, ExitStack() as ctx:
        data = ctx.enter_context(tc.tile_pool(name="data", bufs=1))
        work = ctx.enter_context(tc.tile_pool(name="work", bufs=8))
        sqp = ctx.enter_context(tc.tile_pool(name="sqp", bufs=6))
        consts = ctx.enter_context(tc.tile_pool(name="consts", bufs=1))
        psum = ctx.enter_context(tc.tile_pool(name="psum", bufs=1, space="PSUM"))
        resp = ctx.enter_context(tc.tile_pool(name="resp", bufs=1))

        # SWDGE for the tiny const loads so the HWDGE chunk stream starts at 0
        w_sb = consts.tile([128, 2 * NCH], pr_dt, tag="w", name="w_sb")
        nc.gpsimd.dma_start(w_sb[:], wz[:])
        onesz = consts.tile([128, 2], ss_dt, tag="onesz", name="onesz")
        nc.gpsimd.dma_start(onesz[:], oz[:])
        onesz16 = consts.tile([128, 2], mybir.dt.float16, tag="onesz16",
                              name="onesz16")
        nc.gpsimd.dma_start(onesz16[:], oz16[:])



        ps = [
            psum.tile([2, SLICE], F32, tag=f"ps{s}", name=f"ps{s}")
            for s in range(NSL)
        ]
        ret = resp.tile([2, HALF], F32, tag="ret", name="ret")

        # Slice-major streaming: the shard is DMA'd as T-blocks x 8 D-chunks,
        # so every PSUM slice finishes accumulating (and retires + stores)
        # right after its block's DMAs land. Only the final 512-col sliver's
        # sub->square->matmul->retire chain hangs off the last byte.
        # blocks: three 1024-col blocks, then two 512-col slivers.
        blocks = [(0, 1024), (1024, 1024), (2048, 1024), (3072, 512), (3584, 512)]

        tiles = {}
        for bi, (bo, bw) in enumerate(blocks):
            for c in range(NCH):
                t = data.tile(
                    [128, bw + 1], pr_dt, tag=f"b{bi}c{c}", name=f"b{bi}c{c}"
                )
                nc.sync.dma_start(
                    t[:], ht[c * 128 : (c + 1) * 128, bo : bo + bw + 1]
                )
                tiles[(bi, c)] = t

        ip = 0
        for bi, (bo, bw) in enumerate(blocks):
            last = bi == len(blocks) - 1
            for c in range(NCH):
                t = tiles[(bi, c)]
                tf = t[:].bitcast(F32)
                delta = work.tile([128, bw], F32, tag="delta", name=f"d{bi}_{c}")
                # GPSIMD (otherwise idle) fully handles every 4th piece:
                # sub + square (fp16 out), freeing V and ACT. The phase keeps
                # the final sliver on the fast V/ACT path.
                on_g = ip % 4 == 1
                ip += 1
                sub_eng = nc.gpsimd if on_g else nc.vector
                sub_eng.tensor_tensor(
                    delta[:],
                    tf[:, 1 : 1 + bw],
                    tf[:, 0:bw],
                    mybir.AluOpType.subtract,
                )
                if on_g:
                    sq = sqp.tile([128, bw], mybir.dt.float16, tag="sq",
                                  name=f"sq{bi}_{c}")
                    nc.gpsimd.tensor_tensor(
                        sq[:], delta[:], delta[:], mybir.AluOpType.mult
                    )
                else:
                    sq = sqp.tile([128, bw], ss_dt, tag="sq", name=f"sq{bi}_{c}")
                    nc.scalar.activation(
                        sq[:], delta[:], mybir.ActivationFunctionType.Square
                    )
                for si in range(bw // SLICE):
                    s = (bo + si * SLICE) // SLICE
                    slo = si * SLICE
                    nc.tensor.matmul(
                        ps[s][:],
                        w_sb[:, 2 * c : 2 * c + 2],
                        t[:, 1 + slo : 1 + slo + SLICE],
                        start=(c == 0),
                        stop=False,
                    )
                    nc.tensor.matmul(
                        ps[s][:],
                        onesz16[:] if on_g else onesz[:],
                        sq[:, slo : slo + SLICE],
                        start=False,
                        stop=(c == NCH - 1),
                    )
            # block complete: evacuate its PSUM slices and store them
            for si in range(bw // SLICE):
                s = (bo + si * SLICE) // SLICE
                dst = ret[:, s * SLICE : (s + 1) * SLICE]
                if si % 2 == 0 and not last:
                    nc.scalar.copy(dst, ps[s][:])
                else:
                    nc.vector.tensor_copy(dst, ps[s][:])
            nc.sync.dma_start(out[:, bo : bo + bw], ret[:, bo : bo + bw])

    nc.compile()
    return nc


def _get_nc():
    key = (PRIOR_MODE, SS_MODE)
    if key not in _CACHE:
        _CACHE[key] = _build()
    return _CACHE[key]


def _make_shards(hidden, W):
    wz = np.zeros((128, 2 * NCH), dtype=np.float32)
    for c in range(NCH):
        wz[:, 2 * c] = W[c * 128 : (c + 1) * 128]
    ss_np = {"f16": np.float16, "bf16": None, "f32": np.float32,
             "f32r": np.float32}[SS_MODE]
    oz = np.zeros((128, 2), dtype=ss_np)
    oz[:, 1] = 1.0
    oz16 = oz.astype(np.float16)
    in_maps = []
    for b in range(B):
        for h in range(2):
            start = h * HALF
            prev = hidden[b, max(start - 1, 0)]
            block = hidden[b, start : start + HALF]
            sh = np.empty((D, HALF + 1), dtype=np.float32)
            sh[:, 0] = prev
            sh[:, 1:] = block.T
            in_maps.append({"ht": sh, "wz": wz, "oz": oz, "oz16": oz16})
    return in_maps


def _postprocess(prior_raw, ss, b):
    """float64 host postprocess of the per-timestep reductions."""
    prior_raw = prior_raw.astype(np.float64) + float(b)
    runtime_raw = np.sqrt(ss.astype(np.float64) / D)

    def standardize(v):
        mean = v.mean(axis=1, keepdims=True)
        std = np.maximum(v.std(axis=1, keepdims=True), 1e-6)
        return (v - mean) / std

    def sigmoid(x):
        return 1.0 / (1.0 + np.exp(-x))

    runtime_logits = standardize(runtime_raw)
    runtime_score = sigmoid(runtime_logits)
    prior_logits = standardize(prior_raw)
    prior_score = sigmoid(prior_logits)

    combined = PRIOR_W * prior_logits + RUNTIME_W * runtime_logits
    scores = sigmoid(combined)

    positions = np.arange(T, dtype=np.int32)
    span = np.stack([np.maximum(positions - 1, 0), positions], axis=-1)
    span_bounds = np.broadcast_to(span, (B, T, 2)).astype(np.int32)

    left = np.concatenate([scores[:, :1], scores[:, :-1]], axis=1)
    right = np.concatenate([scores[:, 1:], scores[:, -1:]], axis=1)
    valid = (scores >= left) & (scores >= right) & (scores >= THRESHOLD)

    masked = np.where(valid, scores, -np.inf)
    # jax.lax.top_k: descending values, ties -> lowest index first
    order = np.argsort(-masked, axis=1, kind="stable")[:, :MAX_CAND]
    top_vals = np.take_along_axis(masked, order, axis=1)
    cand_mask = top_vals > -np.inf
    cand_idx = np.sort(np.where(cand_mask, order, T), axis=-1).astype(np.int32)

    return (
        scores.astype(np.float32),
        combined.astype(np.float32),
        prior_score.astype(np.float32),
        runtime_score.astype(np.float32),
        span_bounds,
        cand_idx,
        cand_mask,
    )


def kernel(hidden, W, b):
    global LAST_EXEC_NS, LAST_RESULTS
    hidden = np.asarray(hidden, dtype=np.float32)
    W = np.asarray(W, dtype=np.float32)
    nc = _get_nc()
    in_maps = _make_shards(hidden, W)
    res = run_bass_kernel_spmd(
        nc, in_maps, core_ids=list(range(NCORES)), trace=TRACE
    )
    LAST_EXEC_NS = res.exec_time_ns
    LAST_RESULTS = res

    prior_raw = np.empty((B, T), dtype=np.float32)
    ss = np.empty((B, T), dtype=np.float32)
    for i in range(NCORES):
        bi, h = divmod(i, 2)
        o = res.results[i]["out"]
        prior_raw[bi, h * HALF : (h + 1) * HALF] = o[0]
        ss[bi, h * HALF : (h + 1) * HALF] = o[1]

    return _postprocess(prior_raw, ss, b)
